# revision 66
# baseline (speedup 1.0000x reference)
"""GAT 3-layer Bass kernel for 8 trn2 cores.

v2 design:
- Each core owns a contiguous shard of 6250 dst nodes (49 windows of 128).
- Per layer, each core computes the (rotated) table rows for its OWN nodes
  only, inside the previous layer's window-evacuation path; an AllGather
  shares the full 50176-row table (256B rows) across cores.
- Edge aggregation: per chunk (2 windows), dma_gather fetches per-edge
  256B rows from the shared table; attention weights ef are computed from
  the gathered alpha_src (rotated coord 0) plus a one-hot-matmul scatter of
  the per-window alpha_dst; a one-hot matmul accumulates the softmax
  numerator/denominator per window in PSUM.
- Rotation: T_L = diag(||a_src||,1,..) @ Q_L with Q rows 0/1 spanning
  (a_src, a_dst); table rows are h' = h @ T^T so h'[0] == alpha_src; the
  inverse R = D^{-1} Q is applied per window before relu.
- Edge padding uses trailing -1 indices which the gather ucode pops (no
  descriptor-generation cost on the GPSIMD critical path).
"""
import numpy as np
import concourse.bacc as bacc
import concourse.bass as bass
from concourse import bass_utils
from concourse.tile import TileContext
import concourse.mybir as mybir

N, H, C_OUT, G = 50000, 128, 10, 128
NCORES = 8
NPC = N // NCORES            # 6250
WPC = 49                     # 128-dst windows per core
CHUNK_W = 2
NCHUNK = (WPC + CHUNK_W - 1) // CHUNK_W   # 25
NQUEUES = 4
NGT = 6                      # gather buffer depth
PAD_IDX = 0                  # gather pad index (-1 = popped by ucode)
SHARD_PAD = WPC * 128        # 6272
NPAD = SHARD_PAD * NCORES    # 50176
ROW = 128                    # gather row: 128 f16 = 256B
A_W = 25                     # windows 0..24 -> region A; 25..48 -> region B
A_SZ = A_W * 128             # 3200 own rows in region A
B_SZ = SHARD_PAD - A_SZ      # 3072
A_TOT = NCORES * A_SZ        # 25600 (region A table rows)
B_TOT = NCORES * B_SZ        # 24576
EXP_SHIFT = 4.0

F16, F32, I16 = mybir.dt.float16, mybir.dt.float32, mybir.dt.int16
F8 = mybir.dt.float8e4
AF = mybir.ActivationFunctionType
OP = mybir.AluOpType


def prep_edges(edge_index):
    """Edge partition/packing. Returns (chunks_meta, per-core tensors, sizes).

    chunks_meta[ch] = dict(tg0, tg1, ct, slots=[(tile, window), ...])
    Slot list (tile-major) is uniform across cores; per-core dstl/m0t encode
    each slot's membership. Gather idx arrays carry trailing -1 padding.
    """
    src = np.concatenate([edge_index[0], np.arange(N)]).astype(np.int64)
    dst = np.concatenate([edge_index[1], np.arange(N)]).astype(np.int64)
    sc, sn = src // NPC, src % NPC
    row_id = np.where(sn < A_SZ, sc * A_SZ + sn,
                      A_TOT + sc * B_SZ + (sn - A_SZ))

    groups = {}
    for c in range(NCORES):
        m = (dst // NPC) == c
        r, dl = row_id[m], dst[m] - c * NPC
        win = dl // 128
        for ch in range(NCHUNK):
            wlo, whi = CHUNK_W * ch, min(CHUNK_W * ch + CHUNK_W - 1, WPC - 1)
            inch = (win >= wlo) & (win <= whi)
            for hi in (0, 1):
                mm = inch & ((r >= A_TOT) == bool(hi))
                rr, ww, dd = r[mm], win[mm], dl[mm]
                o = np.lexsort((rr, ww))
                groups[(c, ch, hi)] = (rr[o] - (A_TOT if hi else 0),
                                       ww[o], (dd - ww * 128)[o])

    chunks = []
    idx_arr = {0: [[] for _ in range(NCORES)], 1: [[] for _ in range(NCORES)]}
    dstl_cols = [[] for _ in range(NCORES)]
    m0t_blocks = [[] for _ in range(NCORES)]
    m0f_blocks = [[] for _ in range(NCORES)]
    for ch in range(NCHUNK):
        tg = {}
        for hi in (0, 1):
            mx = max(len(groups[(c, ch, hi)][0]) for c in range(NCORES))
            tg[hi] = max(1, -(-mx // 128))
        slots = []
        for hi in (0, 1):
            base_t = 0 if hi == 0 else tg[0]
            for tl in range(tg[hi]):
                wset = set()
                for c in range(NCORES):
                    ww = groups[(c, ch, hi)][1][tl * 128:(tl + 1) * 128]
                    wset.update(np.unique(ww).tolist())
                if not wset:
                    wset = {CHUNK_W * ch}
                for w in sorted(wset):
                    slots.append((base_t + tl, w))
        for c in range(NCORES):
            for hi in (0, 1):
                rr = groups[(c, ch, hi)][0]
                L = tg[hi] * 128
                ridx = np.full(L, PAD_IDX, np.int64)
                ridx[:len(rr)] = rr
                idx_arr[hi][c].append(ridx)
            for (t, w) in slots:
                hi = 0 if t < tg[0] else 1
                tl = t if hi == 0 else t - tg[0]
                _, ww, dd = groups[(c, ch, hi)]
                wt = ww[tl * 128:(tl + 1) * 128]
                dt_ = dd[tl * 128:(tl + 1) * 128]
                dcol = np.full(128, -1.0, np.float32)
                sel = np.where(wt == w)[0]
                dcol[sel] = dt_[sel]
                dstl_cols[c].append(dcol)
                m0t_blocks[c].append(
                    (np.arange(128)[:, None] == dcol[None, :]))
                m0f_blocks[c].append(
                    (dcol[:, None] == np.arange(128)[None, :]))
        chunks.append(dict(tg0=tg[0], tg1=tg[1], ct=tg[0] + tg[1], slots=slots))

    def wrap16(a):
        a = a.astype(np.int16).reshape(-1, 16).T
        return np.tile(a, (8, 1)).copy()

    cores = []
    for c in range(NCORES):
        lo = np.concatenate(idx_arr[0][c])
        hi = np.concatenate(idx_arr[1][c])
        dstl = np.stack(dstl_cols[c], axis=1).astype(np.float16)  # [128, NSLOT]
        m0t = np.concatenate(m0t_blocks[c], axis=1).astype(
            mybir.dt.np(F8))                                      # [128, NSLOT*128]
        m0f = np.concatenate(m0f_blocks[c], axis=1).astype(
            mybir.dt.np(F8))                                      # [128, NSLOT*128]
        cores.append(dict(idxlo=wrap16(lo), idxhi=wrap16(hi),
                          dstl=dstl, m0t=m0t, m0f=m0f))
    n_lo = sum(len(a) for a in idx_arr[0][0])
    n_hi = sum(len(a) for a in idx_arr[1][0])
    NSLOT = sum(len(ch["slots"]) for ch in chunks)
    return chunks, cores, NSLOT, n_lo, n_hi


def make_weight_inputs(Ws, asrcs, adsts, bs, lin_W, lin_b):
    waug = np.zeros((128, 3, 129), np.float16)
    runr = np.zeros((128, 3, 128), np.float16)
    bcol = np.zeros((128, 3), np.float32)
    for i in range(3):
        W = Ws[i].astype(np.float64)
        a_s = asrcs[i].astype(np.float64)
        a_d = adsts[i].astype(np.float64)
        d0 = np.linalg.norm(a_s)
        q0 = a_s / d0
        v = a_d - (a_d @ q0) * q0
        q1 = v / np.linalg.norm(v)
        Mstack = np.column_stack([q0, q1, np.eye(128)[:, :126]])
        Qf, _ = np.linalg.qr(Mstack)
        if Qf[:, 0] @ q0 < 0:
            Qf[:, 0] *= -1
        if Qf[:, 1] @ q1 < 0:
            Qf[:, 1] *= -1
        Qr = Qf.T                      # rows orthonormal; row0=q0, row1=q1
        T = Qr.copy(); T[0] *= d0      # h' = h @ T^T ; h'[0] = alpha_src
        R = Qr.copy(); R[0] /= d0      # h = h' @ R
        assert np.allclose(T.T @ R, np.eye(128), atol=1e-10)
        waug[:, i, 0:128] = W @ T.T
        waug[:, i, 128] = W @ a_d
        runr[:, i, :] = R
        bcol[:, i] = bs[i]
    iota = np.broadcast_to(np.arange(128, dtype=np.float16), (128, 128)).copy()
    return dict(
        waug=waug, runr=runr, bcol=bcol,
        linw=lin_W.astype(np.float16),
        linb=np.broadcast_to(lin_b.astype(np.float32), (128, C_OUT)).copy(),
        iota=iota, idm=np.eye(128, dtype=np.float16),
    )


def make_xsT0(x, core):
    out = np.zeros((128, SHARD_PAD), np.float16)
    out[:, :NPC] = x[core * NPC:(core + 1) * NPC].astype(np.float16).T
    return out


def make_obt(batch, core):
    """Pool one-hot: obt[n, w*128+g] = 1 iff batch[own node n of window w]==g."""
    obt = np.zeros((128, WPC * 128), np.float32)
    ids = batch[core * NPC:(core + 1) * NPC].astype(np.int64)
    for i, g in enumerate(ids):
        w, n = i // 128, i % 128
        obt[n, w * 128 + g] = 1.0
    return obt.astype(mybir.dt.np(F8))


def make_batch_input(batch, core):
    bl = np.full((128, WPC), -1.0, np.float32)
    ids = batch[core * NPC:(core + 1) * NPC].astype(np.float32)
    for w in range(WPC):
        seg = ids[w * 128:(w + 1) * 128]
        bl[: len(seg), w] = seg
    return bl


def split_waits(nc, maxw=1):
    n = 0
    for func in nc.m.functions:
        for block in func.blocks:
            new = []
            for inst in block.instructions:
                si = inst.sync_info
                if si is not None and si.on_wait and len(si.on_wait) > maxw:
                    w = list(si.on_wait); extra, keep = w[:-maxw], w[-maxw:]
                    while extra:
                        ck, extra = extra[:maxw], extra[maxw:]
                        new.append(mybir.InstNoOp(name=f"ws-{n}", engine=inst.engine,
                            sync_info=mybir.SyncInfo(on_wait=ck, on_update=[])))
                        n += 1
                    si.on_wait = keep
                new.append(inst)
            block.instructions = new
    return n


def build(nc, chunks, NSLOT, n_lo, n_hi, n_layers=3, edge_mode=3,
          with_pool=True, dump_htab=0):
    CT_MAX = max(c["ct"] for c in chunks)
    NS_MAX = max(len(c["slots"]) for c in chunks)

    waug_in = nc.dram_tensor("waug", [128, 3, 129], F16, kind="ExternalInput")
    runr_in = nc.dram_tensor("runr", [128, 3, 128], F16, kind="ExternalInput")
    bcol_in = nc.dram_tensor("bcol", [128, 3], F32, kind="ExternalInput")
    linw_in = nc.dram_tensor("linw", [128, C_OUT], F16, kind="ExternalInput")
    linb_in = nc.dram_tensor("linb", [128, C_OUT], F32, kind="ExternalInput")
    iota_in = nc.dram_tensor("iota", [128, 128], F16, kind="ExternalInput")
    idm_in  = nc.dram_tensor("idm", [128, 128], F16, kind="ExternalInput")
    bl_in   = nc.dram_tensor("batchl", [128, WPC], F32, kind="ExternalInput")
    ilo_in  = nc.dram_tensor("idxlo", [128, n_lo // 16], I16, kind="ExternalInput")
    ihi_in  = nc.dram_tensor("idxhi", [128, n_hi // 16], I16, kind="ExternalInput")
    dstl_in = nc.dram_tensor("dstl", [128, NSLOT], F16, kind="ExternalInput")
    m0t_in  = nc.dram_tensor("m0t", [128, NSLOT * 128], F8, kind="ExternalInput")
    m0f_in  = nc.dram_tensor("m0f", [128, NSLOT * 128], F8, kind="ExternalInput")
    xsT0_in = nc.dram_tensor("xsT0", [128, SHARD_PAD], F16, kind="ExternalInput")
    obt_in  = nc.dram_tensor("obt", [128, WPC * 128], F8, kind="ExternalInput")
    out_t   = nc.dram_tensor("out", [G, C_OUT], F32, kind="ExternalOutput")
    htab_out = (nc.dram_tensor("htab_out", [dump_htab, ROW], F16,
                               kind="ExternalOutput") if dump_htab else None)

    with TileContext(nc) as tc:
        with tc.tile_pool(name="const", bufs=1) as constp, \
             tc.tile_pool(name="gath", bufs=1) as gathp, \
             tc.tile_pool(name="m0fp", bufs=2) as m0fp, \
             tc.tile_pool(name="rhsp", bufs=2) as rhsp, \
             tc.tile_pool(name="ewp", bufs=2) as ewp, \
             tc.tile_pool(name="evac", bufs=3) as evp, \
             tc.tile_pool(name="stage", bufs=3) as stp, \
             tc.tile_pool(name="m0tp", bufs=2) as m0tp, \
             tc.tile_pool(name="psw", bufs=3, space="PSUM") as psw, \
             tc.tile_pool(name="psadx", bufs=1, space="PSUM") as psadx, \
             tc.tile_pool(name="pstr", bufs=1, space="PSUM") as pstr, \
             tc.tile_pool(name="psunx", bufs=2, space="PSUM") as psunx, \
             tc.tile_pool(name="psp", bufs=1, space="PSUM") as psp, \
             tc.tile_pool(name="dram", bufs=1, space="DRAM") as dram:

            waug = constp.tile([128, 3, 129], F16)
            runr = constp.tile([128, 3, 128], F16)
            bcol = constp.tile([128, 3], F32)
            linw = constp.tile([128, C_OUT], F16)
            linb = constp.tile([128, C_OUT], F32)
            iota = constp.tile([128, 128], F16)
            idm  = constp.tile([128, 128], F16)
            bl   = constp.tile([128, WPC], F32)
            ilo  = constp.tile([128, n_lo // 16], I16)
            ihi  = constp.tile([128, n_hi // 16], I16)
            dstl = constp.tile([128, NSLOT], F16)
            xsT0 = constp.tile([128, SHARD_PAD], F16)
            obt  = constp.tile([128, WPC * 128], F8)
            for t, s in [(xsT0, xsT0_in), (waug, waug_in), (runr, runr_in),
                         (bcol, bcol_in), (linw, linw_in), (linb, linb_in),
                         (iota, iota_in), (idm, idm_in), (bl, bl_in),
                         (ilo, ilo_in), (ihi, ihi_in), (dstl, dstl_in),
                         (obt, obt_in)]:
                nc.sync.dma_start(out=t[:], in_=s[:])

            negshift = constp.tile([128, 1], F32)
            nc.vector.memset(negshift[:], -EXP_SHIFT)
            eps = constp.tile([128, 1], F32, name="eps")
            nc.vector.memset(eps[:], 1e-6)
            adl = [constp.tile([128, WPC], F16, name=f"adl{i}") for i in range(2)]

            gt_bufs = [gathp.tile([128, CT_MAX, ROW], F16, name=f"gt{i}",
                                  tag=f"gt{i}") for i in range(NGT)]
            for gtb in gt_bufs:
                nc.vector.memset(gtb[:], 0.0)

            hownA = [dram.tile([A_SZ, ROW], F16, name=f"hownA{i}",
                               tag=f"hownA{i}") for i in range(2)]
            hownB = [dram.tile([B_SZ, ROW], F16, name=f"hownB{i}",
                               tag=f"hownB{i}") for i in range(2)]
            hshA = [dram.tile([NCORES, A_SZ, ROW], F16, addr_space="Shared",
                              name=f"hshA{i}", tag=f"hshA{i}") for i in range(3)]
            hshB = [dram.tile([NCORES, B_SZ, ROW], F16, addr_space="Shared",
                              name=f"hshB{i}", tag=f"hshB{i}") for i in range(3)]
            locA = [dram.tile([NCORES, A_SZ, ROW], F16, name=f"locA{i}",
                              tag=f"locA{i}") for i in range(2)]
            locB = [dram.tile([NCORES, B_SZ, ROW], F16, name=f"locB{i}",
                              tag=f"locB{i}") for i in range(2)]
            pool_bi = dram.tile([128, 129], F32)
            pool_bo = dram.tile([128, 129], F32, addr_space="Shared")

            # ---- produce own-shard table rows for table `ti` from y [f, n] ----
            def own_rows(ti, w, y_ap):
                ps = psunx.tile([128, 129], F32, tag="unx")
                nc.tensor.matmul(ps[:], y_ap, waug[:, ti, :], start=True,
                                 stop=True, skip_group_check=True)
                st = stp.tile([128, 128], F16, tag="st")
                nc.scalar.activation(st[:], ps[:, 0:128], AF.Copy)
                nc.scalar.activation(adl[ti % 2][:, w:w + 1], ps[:, 128:129],
                                     AF.Copy)
                if w < A_W:
                    dst_ap = hownA[ti % 2][w * 128:(w + 1) * 128, :]
                else:
                    dst_ap = hownB[ti % 2][(w - A_W) * 128:(w - A_W + 1) * 128, :]
                nc.sync.dma_start(
                    out=dst_ap.rearrange("(b p) e -> p b e", p=128),
                    in_=st[:].unsqueeze(1))

            def allgatherA(ti):
                nc.gpsimd.collective_compute(
                    "AllGather", OP.bypass, replica_groups=[list(range(NCORES))],
                    ins=[hownA[ti % 2][:].opt()], outs=[hshA[ti][:].opt()])
                nc.sync.dma_start(out=locA[ti % 2][:], in_=hshA[ti][:])

            def allgatherB(ti):
                nc.gpsimd.collective_compute(
                    "AllGather", OP.bypass, replica_groups=[list(range(NCORES))],
                    ins=[hownB[ti % 2][:].opt()], outs=[hshB[ti][:].opt()])
                nc.sync.dma_start(out=locB[ti % 2][:], in_=hshB[ti][:])

            pool_ps = psp.tile([128, 129], F32, tag="pool", name="pool_ps")

            # ---- per-window output path for edge layer `layer` ----
            def window_out(layer, w, ps):
                dn = evp.tile([128, 1], F32, tag="dn")
                nc.scalar.activation(dn[:], ps[:, 128:129], AF.Relu, bias=eps[:])
                rc = evp.tile([128, 1], F32, tag="rc")
                nc.vector.reciprocal(rc[:], dn[:])
                xw = evp.tile([128, 128], F16, tag="xw")
                nc.scalar.activation(xw[:], ps[:, 0:128], AF.Copy, scale=rc[:])
                tp = pstr.tile([128, 128], F16, tag="tr")
                nc.tensor.transpose(tp[:], xw[:], idm[:])
                xwT = evp.tile([128, 128], F16, tag="xwT")
                nc.vector.tensor_copy(xwT[:], tp[:])
                up = psunx.tile([128, 129], F32, tag="unx", name=f"u_{layer}_{w}")
                nc.tensor.matmul(up[:, 0:128], runr[:, layer, :], xwT[:],
                                 start=True, stop=True, skip_group_check=True)
                y = evp.tile([128, 128], F16, tag="y")
                nc.scalar.activation(y[:], up[:, 0:128], AF.Relu,
                                     bias=bcol[:, layer:layer + 1])
                if layer < 2:
                    own_rows(layer + 1, w, y[:])
                else:
                    tp2 = pstr.tile([128, 128], F16, tag="tr",
                                    name=f"tr2_{w}")
                    nc.tensor.transpose(tp2[:], y[:], idm[:])
                    x1 = evp.tile([128, 129], F16, tag="x1")
                    nc.vector.tensor_copy(x1[:, 0:128], tp2[:])
                    nc.vector.memset(x1[:, 128:129], 1.0)
                    nc.tensor.matmul(pool_ps[:], obt[:, w * 128:(w + 1) * 128],
                                     x1[:],
                                     start=(w == 0), stop=(w == WPC - 1),
                                     skip_group_check=True)

            # ---- edge phase ----
            offA = [0]
            offB = [0]
            for meta in chunks:
                offA.append(offA[-1] + meta["tg0"] * 128)
                offB.append(offB[-1] + meta["tg1"] * 128)

            def issue_gather(layer, ch, hi):
                meta = chunks[ch]
                tg = meta["tg1"] if hi else meta["tg0"]
                n_seg = tg * 128
                gt = gt_bufs[(layer * NCHUNK + ch) % NGT]
                t0 = meta["tg0"] if hi else 0
                loc = locB[layer % 2] if hi else locA[layer % 2]
                src_ap = loc[:].rearrange("r s e -> (r s) e")
                o = (offB if hi else offA)[ch]
                idxs = (ihi if hi else ilo)[:, o // 16:(o + n_seg) // 16]
                nc.gpsimd.dma_gather(
                    out_ap=gt[:, t0:t0 + tg, :], in_ap=src_ap,
                    idxs_ap=idxs, num_idxs=n_seg, num_idxs_reg=n_seg,
                    elem_size=ROW, single_packet=False,
                    queue_num=nc._gq[0] % NQUEUES)
                nc._gq[0] += 1

            def edge_phase(layer, agb=None):
                adl_cur = adl[layer % 2]
                slot_off = 0
                # prefetch region-A gathers of the first chunks so they run
                # while the region-B AllGather still waits for its inputs
                npre = 5 if agb is not None else 0
                for ch in range(npre):
                    issue_gather(layer, ch, 0)
                if agb is not None:
                    agb()
                for ch, meta in enumerate(chunks):
                    ct, tg0, tg1 = meta["ct"], meta["tg0"], meta["tg1"]
                    slots = meta["slots"]; ns = len(slots)
                    gt = gt_bufs[(layer * NCHUNK + ch) % NGT]
                    if ch >= npre:
                        issue_gather(layer, ch, 0)
                    issue_gather(layer, ch, 1)
                    if edge_mode < 1:
                        slot_off += ns
                        continue
                    m0t = m0tp.tile([128, NS_MAX * 128], F8, tag="m0t")
                    nc.sync.dma_start(
                        out=m0t[:, 0:ns * 128],
                        in_=m0t_in[:, slot_off * 128:(slot_off + ns) * 128])
                    adx = psadx.tile([128, CT_MAX], F32, tag="adx")
                    tile_slots = {}
                    for si, (t, w) in enumerate(slots):
                        tile_slots.setdefault(t, []).append(si)
                    for t, sis in tile_slots.items():
                        for k, si in enumerate(sis):
                            _, w = slots[si]
                            nc.tensor.matmul(
                                adx[:, t:t + 1], m0t[:, si * 128:(si + 1) * 128],
                                adl_cur[:, w:w + 1], start=(k == 0),
                                stop=(k == len(sis) - 1), skip_group_check=True)
                    z  = ewp.tile([128, CT_MAX], F32, tag="z")
                    e1 = ewp.tile([128, CT_MAX], F32, tag="e1")
                    ef = ewp.tile([128, CT_MAX], F32, tag="ef")
                    nc.vector.tensor_tensor(z[:, 0:ct].unsqueeze(2),
                                            gt[:, 0:ct, 0:1],
                                            adx[:, 0:ct].unsqueeze(2), OP.add)
                    nc.scalar.activation(e1[:, 0:ct], z[:, 0:ct], AF.Exp,
                                         bias=negshift[:])
                    nc.scalar.activation(z[:, 0:ct], z[:, 0:ct], AF.Exp,
                                         bias=negshift[:], scale=0.2)
                    nc.vector.tensor_tensor(ef[:, 0:ct], e1[:, 0:ct],
                                            z[:, 0:ct], OP.max)
                    rhs = rhsp.tile([128, CT_MAX, 130], F16, tag="rhs")
                    nc.vector.tensor_tensor(
                        rhs[:, 0:ct, 0:128], gt[:, 0:ct, :],
                        ef[:, 0:ct].unsqueeze(2).to_broadcast((128, ct, 128)),
                        OP.mult)
                    nc.scalar.activation(rhs[:, 0:ct, 128:129],
                                         ef[:, 0:ct].unsqueeze(2), AF.Copy)
                    if edge_mode < 2:
                        slot_off += ns
                        continue
                    m0 = m0fp.tile([128, NS_MAX * 128], F8, tag="m0f")
                    nc.sync.dma_start(
                        out=m0[:, 0:ns * 128],
                        in_=m0f_in[:, slot_off * 128:(slot_off + ns) * 128])
                    wf, wl = {}, {}
                    for si, (t, w) in enumerate(slots):
                        if w not in wf:
                            wf[w] = si
                        wl[w] = si
                    psws = {w: psw.tile([128, 129], F32, tag="win",
                                        name=f"win{layer}_{ch}_{w}")
                            for w in wf}
                    for si, (t, w) in enumerate(slots):
                        nc.tensor.matmul(psws[w][:],
                                         m0[:, si * 128:(si + 1) * 128],
                                         rhs[:, t, 0:129],
                                         start=(si == wf[w]),
                                         stop=(si == wl[w]),
                                         skip_group_check=True)
                    if edge_mode >= 3:
                        for w in sorted(wf):
                            window_out(layer, w, psws[w])
                    slot_off += ns
                    if (edge_mode >= 3 and layer < 2
                            and ch == (A_W - 1) // CHUNK_W):
                        allgatherA(layer + 1)

            # ================= main =================
            for w in range(A_W):
                own_rows(0, w, xsT0[:, w * 128:(w + 1) * 128])
            allgatherA(0)
            for w in range(A_W, WPC):
                own_rows(0, w, xsT0[:, w * 128:(w + 1) * 128])
            for layer in range(n_layers):
                if layer == 0 or edge_mode >= 3:
                    edge_phase(layer, agb=lambda ti=layer: allgatherB(ti))
                else:
                    edge_phase(layer)
            if dump_htab:
                hcp = stp.tile([128, dump_htab // 128, ROW], F16, tag="hcp")
                nc.sync.dma_start(
                    out=hcp[:],
                    in_=locA[0][:].rearrange("r s e -> (r s) e")
                        [0:dump_htab, :].rearrange("(b p) e -> p b e", p=128))
                nc.sync.dma_start(
                    out=htab_out[:].rearrange("(b p) e -> p b e", p=128),
                    in_=hcp[:])
            if not with_pool or n_layers < 3 or edge_mode < 3:
                zz = evp.tile([128, C_OUT], F32, tag="res")
                nc.vector.memset(zz[:], 0.0)
                nc.sync.dma_start(out=out_t[:], in_=zz[:])
                return nc

            # ---- pool + final linear ----
            pooled = evp.tile([128, 129], F32, tag="pooled")
            nc.vector.tensor_copy(pooled[:], pool_ps[:])
            nc.sync.dma_start(out=pool_bi[:], in_=pooled[:])
            nc.gpsimd.collective_compute(
                "AllReduce", OP.add, replica_groups=[list(range(NCORES))],
                ins=[pool_bi[:].opt()], outs=[pool_bo[:].opt()])
            nc.sync.dma_start(out=pooled[:], in_=pool_bo[:])
            cnt = evp.tile([128, 1], F32, tag="cnt")
            nc.vector.tensor_scalar_max(cnt[:], pooled[:, 128:129], 1.0)
            rcn = evp.tile([128, 1], F32, tag="rcn")
            nc.vector.reciprocal(rcn[:], cnt[:])
            pm = evp.tile([128, 128], F16, tag="pm")
            nc.scalar.activation(pm[:], pooled[:, 0:128], AF.Copy, scale=rcn[:])
            pt = pstr.tile([128, 128], F16, tag="tr")
            nc.tensor.transpose(pt[:], pm[:], idm[:])
            pts = evp.tile([128, 128], F16, tag="pts")
            nc.vector.tensor_copy(pts[:], pt[:])
            ho = psw.tile([128, 129], F32, tag="win", name="ho")
            nc.tensor.matmul(ho[:, 0:C_OUT], pts[:], linw[:], start=True,
                             stop=True, skip_group_check=True)
            res = evp.tile([128, C_OUT], F32, tag="res")
            nc.vector.tensor_tensor(res[:], ho[:, 0:C_OUT], linb[:], OP.add)
            nc.sync.dma_start(out=out_t[:], in_=res[:])
    return nc


def run(inputs, trace=False, **build_kw):
    chunks, cores, NSLOT, n_lo, n_hi = prep_edges(np.asarray(inputs["edge_index"]))
    const_ins = make_weight_inputs(
        [np.asarray(inputs[f"W{i}"]) for i in (1, 2, 3)],
        [np.asarray(inputs[f"a_src{i}"]) for i in (1, 2, 3)],
        [np.asarray(inputs[f"a_dst{i}"]) for i in (1, 2, 3)],
        [np.asarray(inputs[f"b{i}"]) for i in (1, 2, 3)],
        np.asarray(inputs["lin_W"]), np.asarray(inputs["lin_b"]))
    batch = np.asarray(inputs["batch"])
    x = np.asarray(inputs["x"])

    nc = bacc.Bacc("TRN2", target_bir_lowering=False, debug=False,
                   num_devices=NCORES, num_swdge_queues=NQUEUES)
    nc._gq = [0]
    build(nc, chunks, NSLOT, n_lo, n_hi, **build_kw)
    nc.compile()
    split_waits(nc)

    in_maps = []
    for c in range(NCORES):
        m = dict(const_ins)
        m["batchl"] = make_batch_input(batch, c)
        m["obt"] = make_obt(batch, c)
        m["xsT0"] = make_xsT0(x, c)
        m["idxlo"] = cores[c]["idxlo"]
        m["idxhi"] = cores[c]["idxhi"]
        m["dstl"] = cores[c]["dstl"]
        m["m0t"] = cores[c]["m0t"]
        m["m0f"] = cores[c]["m0f"]
        in_maps.append(m)
    res = bass_utils.run_bass_kernel_spmd(nc, in_maps,
                                          core_ids=list(range(NCORES)),
                                          trace=trace)
    return res.results[0], res


def kernel(**inputs):
    """Harness entry: full unsharded inputs -> [128, 10] fp32 output."""
    out, _ = run(inputs)
    if isinstance(out, dict):
        out = out["out"]
    return np.asarray(out, dtype=np.float32)
